# revision 1
# baseline (speedup 1.0000x reference)
"""Trainium2 Bass kernel for nn_CiBabyMambaHar (CI bidirectional Mamba HAR).

Self-contained: host-side weight prep (numpy) + Bass/Tile kernel builder +
SPMD runner over 8 NeuronCores (pure data parallel over batch).
"""
import numpy as np
import ml_dtypes

B, C, T = 256, 9, 128
D, S, NL, EXP, DTR, DCONV = 24, 16, 4, 2, 2, 4
DI = EXP * D  # 48
STEMK, PK, PS = 5, 16, 4
L = (T - PK) // PS + 1  # 29
NCLS = 6
EPS = 1e-5

NCORES = 8
NB = B // NCORES          # 32 batch rows per core
N = NB * C                # 288 sequences per core
NTOK = N * L              # 8352 tokens
CH = 16                   # seqs per matmul chunk
NCH = N // CH             # 18 chunks
CW = CH * L               # 464 chunk width
NT = 9                    # scan tiles (32 seqs each)
SN = 32                   # seqs per scan tile
DP = 4                    # d-groups per scan tile row block
DD = DI // DP             # 12 d per group
HSW = 136                 # stem buffer width per seq
PADW = L + 6              # 35
TP = 96                   # token-par partitions for LN stats
TG = NTOK // TP           # 87
SGRP = 48                 # seqs per stem group
NSG = N // SGRP           # 6


def prep_weights(w):
    g = lambda k: np.asarray(w[k], np.float32)
    bf = ml_dtypes.bfloat16
    p = {}
    bs = g("stem_bn_g") / np.sqrt(g("stem_bn_v") + EPS)
    stem_w = g("stem_w")[:, 0, :] * bs[:, None]
    p["stemW"] = stem_w.T.astype(bf)                                   # [5,24]
    p["stem_b"] = (g("stem_bn_b") - g("stem_bn_m") * bs).astype(np.float32)[:, None]
    pbs = g("patch_bn_g") / np.sqrt(g("patch_bn_v") + EPS)
    W_pp = g("pp_w")[:, :, 0] * pbs[:, None]
    b_patch = g("patch_bn_b") - g("patch_bn_m") * pbs
    pd = g("pd_w")[:, 0, :]
    patchW = np.stack([(W_pp * pd[None, :, j]).T for j in range(PK)], 0)
    p["patchW"] = patchW.transpose(1, 0, 2).reshape(D, PK * D).astype(bf)
    p["patch_bias"] = (b_patch[:, None] + g("pos_embed")[0].T).astype(bf)
    A_chk = np.exp(g("A_log"))
    assert np.allclose(A_chk, np.broadcast_to(
        np.arange(1, S + 1, dtype=np.float32), (NL, DI, S)), atol=1e-3)
    for i in range(NL):
        in_w = g("in_w")[i]
        W_u, W_z = in_w[:DI], in_w[DI:]
        cw = g("conv_w")[i][:, 0, :]
        g1 = g("ln1_g")[i]; b1 = g("ln1_b")[i]
        assert np.all(np.abs(g1) > 1e-6)
        p[f"c1_{i}"] = (b1 / g1).astype(np.float32)[:, None]
        Wc = np.stack([(W_u * cw[:, k:k + 1]).T * g1[:, None] for k in range(DCONV)], 0)
        p[f"Wconv_{i}"] = Wc.transpose(1, 0, 2).reshape(D, DCONV * DI).astype(bf)
        p[f"conv_b_{i}"] = g("conv_b")[i].astype(np.float32)[:, None]
        p[f"Wz_{i}"] = (W_z.T * g1[:, None]).astype(bf)
        xp = g("xproj_w")[i]
        W_dtc = xp[:DTR, :].T @ g("dt_w")[i].T
        p[f"Waug_{i}"] = np.concatenate(
            [W_dtc, np.zeros((DI, S), np.float32),
             xp[DTR:DTR + S].T, xp[DTR + S:].T], 1).astype(bf)  # [48,96] dt|pad|B|C
        p[f"dt_b_{i}"] = g("dt_b")[i].astype(np.float32)[:, None]
        Dp = g("Dp")[i]; out_w = g("out_w")[i]
        p[f"Wouty_{i}"] = out_w.T.astype(bf)                            # [48,24]
        p[f"Woutuc_{i}"] = (out_w * Dp[None, :]).T.astype(bf)           # [48,24]
        g2 = g("ln2_g")[i]; b2 = g("ln2_b")[i]
        p[f"ln2A_{i}"] = np.stack([g2, 0 * g2, 0 * g2], 0).astype(bf)   # [3,24]
        p[f"ln2B_{i}"] = np.stack([0 * g2, g2, -b2], 0).astype(bf)
    one = np.ones(D, np.float32); zero = np.zeros(D, np.float32)
    p["lnI_A"] = np.stack([one, zero, zero], 0).astype(bf)
    p["lnI_B"] = np.stack([zero, one, zero], 0).astype(bf)
    p["ones2a"] = np.stack([one, zero], 1).astype(bf)                   # [24,2]
    p["ones2b"] = np.stack([zero, one], 1).astype(bf)
    p["attnW"] = g("attn_w").T.astype(bf)
    p["attn_b"] = g("attn_b").astype(np.float32)[:, None]
    p["ctx"] = g("ctx").astype(bf)[:, None]
    hg = g("head_g"); hb = g("head_b")
    p["headW"] = (g("head_w") * hg[None, :]).T.astype(bf)               # [24,6]
    p["head_bias"] = (g("head_bias") + g("head_w") @ hb).astype(np.float32)[:, None]
    return p


WSPEC = {
    "stemW": ([STEMK, D], "bf"), "stem_b": ([D, 1], "f32"),
    "patchW": ([D, PK * D], "bf"), "patch_bias": ([D, L], "bf"),
    "lnI_A": ([3, D], "bf"), "lnI_B": ([3, D], "bf"),
    "ones2a": ([D, 2], "bf"), "ones2b": ([D, 2], "bf"),
    "attnW": ([D, D], "bf"), "attn_b": ([D, 1], "f32"), "ctx": ([D, 1], "bf"),
    "headW": ([D, NCLS], "bf"), "head_bias": ([NCLS, 1], "f32"),
}
for _i in range(NL):
    WSPEC.update({
        f"c1_{_i}": ([D, 1], "f32"),
        f"Wconv_{_i}": ([D, DCONV * DI], "bf"), f"conv_b_{_i}": ([DI, 1], "f32"),
        f"Wz_{_i}": ([D, DI], "bf"), f"Waug_{_i}": ([DI, 96], "bf"),
        f"dt_b_{_i}": ([DI, 1], "f32"),
        f"Wouty_{_i}": ([DI, D], "bf"), f"Woutuc_{_i}": ([DI, D], "bf"),
        f"ln2A_{_i}": ([3, D], "bf"), f"ln2B_{_i}": ([3, D], "bf"),
    })


def build(dbg=None):
    import concourse.bacc as bacc
    import concourse.tile as tile
    import concourse.bass as bass
    from concourse import mybir
    import contextlib

    F32, BF16 = mybir.dt.float32, mybir.dt.bfloat16
    A = mybir.AluOpType
    AF = mybir.ActivationFunctionType
    AX = mybir.AxisListType

    nc = bacc.Bacc("TRN2", target_bir_lowering=False, debug=False,
                   num_devices=NCORES)
    xin = nc.dram_tensor("x", [NB, T, C], F32, kind="ExternalInput")
    yout = nc.dram_tensor("y", [NCLS, NB], F32, kind="ExternalOutput")
    WN = {k: nc.dram_tensor(k, shp, BF16 if d == "bf" else F32, kind="ExternalInput")
          for k, (shp, d) in WSPEC.items()}
    dbg = dbg or {}
    dbg_t = {name: nc.dram_tensor(name, shp, BF16 if d == "bf" else F32,
                                  kind="ExternalOutput")
             for name, (shp, d) in dbg.items()}

    with tile.TileContext(nc) as tc:
        ctx = contextlib.ExitStack()
        with ctx:
            W = ctx.enter_context(tc.tile_pool(name="wts", bufs=1))
            per = ctx.enter_context(tc.tile_pool(name="per", bufs=1))
            psp = ctx.enter_context(tc.tile_pool(name="psum", bufs=8, space="PSUM"))
            dram = ctx.enter_context(tc.tile_pool(name="dram", bufs=1, space="DRAM"))

            wt = {}
            for k, (shp, d) in WSPEC.items():
                wt[k] = W.tile(shp, BF16 if d == "bf" else F32, tag=k, name="w_" + k)
                nc.sync.dma_start(wt[k][:], WN[k][:])

            epsT = per.tile([128, 1], F32, tag="epsT")
            nc.vector.memset(epsT[:], EPS)
            oneT = per.tile([128, 1], F32, tag="oneT")
            nc.vector.memset(oneT[:], 1.0)
            hres = per.tile([D, N, L], F32, tag="hres")
            hres_f = hres[:].rearrange("d n p -> d (n p)")
            hnpad = per.tile([D, N, PADW], BF16, tag="hnpad")
            ucF = per.tile([DI, NTOK], BF16, tag="ucF")
            ucB = per.tile([DI, NTOK], BF16, tag="ucB")
            szt = per.tile([DI, NTOK], BF16, tag="szt")
            ddt = dram.tile([2 * DI, NTOK], BF16, tag="ddt")
            ddtu = dram.tile([2 * DI, NTOK], BF16, tag="ddtu")
            dBC = dram.tile([4 * S, NTOK], BF16, tag="dBC")
            dy = dram.tile([2 * DI, NTOK], BF16, tag="dy")
            dst2 = dram.tile([2, NTOK], F32, tag="dst2")
            dst3 = dram.tile([3, NTOK], BF16, tag="dst3")

            def slotA():
                return lay.tile([DI, NTOK], BF16, tag="slotA", name="slotA")

            def slotB():
                return lay.tile([DI, NTOK], BF16, tag="slotB", name="slotB")

            def hsc():
                return lay.tile([DI, NTOK], BF16, tag="hsc", name="hsc")

            def dbg_dump(name, src_ap):
                if name in dbg_t:
                    nc.sync.dma_start(dbg_t[name][:], src_ap)

            # ================= stem + patch =================
            with tc.tile_pool(name="stem", bufs=2) as stp:
                xt16 = stp.tile([T, N], BF16, tag="xt16")
                dbg_has_stem = True
                xtok = stp.tile([T, N], F32, tag="xtok")
                nc.sync.dma_start(
                    xtok[:].rearrange("t (b c) -> t b c", b=NB),
                    bass.AP(tensor=xin, offset=0,
                            ap=[[C, T], [T * C, NB], [1, C]]))
                nc.vector.tensor_copy(xt16[:], xtok[:])
                dbg_dump("d_xt", xt16[:])
                for grp in range(NSG):
                    n0 = grp * SGRP
                    r5 = stp.tile([STEMK, 132, SGRP], BF16, tag="r5")
                    nc.vector.memset(r5[:, 0:2, :], 0.0)
                    nc.vector.memset(r5[:, 126:128, :], 0.0)
                    for k in range(STEMK):
                        c0, c1_ = max(0, 2 - k), min(132, 130 - k)
                        # r5[k, c, n] = xs[n0+n, c+k-2]
                        nc.sync.dma_start(
                            r5[k:k + 1, c0:c1_, :],
                            xt16[c0 + k - 2:c1_ + k - 2, n0:n0 + SGRP])

                    hsb = stp.tile([D, SGRP, HSW], BF16, tag="hsb")
                    if grp == 0:
                        dbg_dump("d_r5", r5[:])
                    nc.vector.memset(hsb[:, :, 0:4], 0.0)
                    nc.vector.memset(hsb[:, :, 132:136], 0.0)
                    for g3 in range(SGRP // 3):
                        pst = psp.tile([80, CW], F32, tag="ps")
                        nc.tensor.matmul(
                            pst[0:D, 0:384],
                            wt["stemW"][:],
                            r5[:, 0:128, 3 * g3:3 * (g3 + 1)],
                            start=True, stop=True)
                        nc.scalar.activation(
                            hsb[:, 3 * g3:3 * (g3 + 1), 4:132].transpose([0, 2, 1]),
                            pst[0:D, 0:384].rearrange("d (c n) -> d c n", n=3),
                            AF.Silu, bias=wt["stem_b"][:], scale=1.0)
                    if grp == 0:
                        dbg_dump("d_hsb", hsb[:])
                    for cc in range(SGRP // CH):
                        c = grp * (SGRP // CH) + cc
                        psq = psp.tile([80, CW], F32, tag="ps")
                        for j in range(PK):
                            m, ph = divmod(j, 4)
                            nc.tensor.matmul(
                                psq[0:D, :], wt["patchW"][:, j * D:(j + 1) * D],
                                hsb[:, CH * cc:CH * (cc + 1), 4 * m + ph:4 * m + ph + 4 * L:4],
                                start=(j == 0), stop=(j == PK - 1))
                        nc.vector.tensor_tensor(
                            hres[:, CH * c:CH * (c + 1), :],
                            psq[0:D, :].rearrange("d (n p) -> d n p", n=CH),
                            wt["patch_bias"][:].unsqueeze(1).broadcast_to([D, CH, L]),
                            A.add)
            dbg_dump("d_h0", hres_f)
            lay = ctx.enter_context(tc.tile_pool(name="lay", bufs=1))
            sc = ctx.enter_context(tc.tile_pool(name="scan", bufs=2))
            sm = ctx.enter_context(tc.tile_pool(name="small", bufs=2))
            sm1 = ctx.enter_context(tc.tile_pool(name="small1", bufs=1))

            # ================= helpers =================
            def ln_stats(src_f32):
                """stats of src [D, NTOK] fp32 -> dst3 = [rho, mu*rho, 1] bf16 in DRAM;
                also fills h16c chunks (bf16 copy of src) returned as full tile."""
                h16 = hsc()
                nc.scalar.activation(h16[0:D, :], src_f32, AF.Copy, scale=1.0)
                for c in range(NCH):
                    sl = slice(CW * c, CW * (c + 1))
                    hsqc = sm.tile([D, CW], BF16, tag="hsqc")
                    nc.scalar.square(hsqc[:], src_f32[:, sl])
                    psS = psp.tile([80, CW], F32, tag="ps")
                    nc.tensor.matmul(psS[0:2, :], wt["ones2a"][:], h16[0:D, sl],
                                     start=True, stop=False)
                    nc.tensor.matmul(psS[0:2, :], wt["ones2b"][:], hsqc[:],
                                     start=False, stop=True)
                    stc = sm.tile([2, CW], F32, tag="stc", bufs=1)
                    nc.vector.tensor_copy(stc[:], psS[0:2, :])
                    nc.sync.dma_start(dst2[:, sl], stc[:])
                tpsA = sm1.tile([TP, TG], F32, tag="tpsA")
                tpsB = sm1.tile([TP, TG], F32, tag="tpsB")
                nc.sync.dma_start(
                    tpsA[:], bass.AP(tensor=dst2.tensor, offset=0,
                                     ap=[[TG, TP], [1, TG]]))
                nc.sync.dma_start(
                    tpsB[:], bass.AP(tensor=dst2.tensor, offset=NTOK,
                                     ap=[[TG, TP], [1, TG]]))
                mu = sm1.tile([TP, TG], F32, tag="mu")
                var = sm1.tile([TP, TG], F32, tag="var")
                t1 = sm1.tile([TP, TG], F32, tag="t1")
                nc.scalar.mul(mu[:], tpsA[:], 1.0 / D)
                nc.scalar.mul(var[:], tpsB[:], 1.0 / D)
                nc.vector.tensor_tensor(t1[:], mu[:], mu[:], A.mult)
                nc.vector.tensor_tensor(var[:], var[:], t1[:], A.subtract)
                nc.scalar.activation(var[:], var[:], AF.Sqrt, bias=epsT[0:TP, :], scale=1.0)
                nc.vector.reciprocal(var[:], var[:])
                nc.vector.tensor_tensor(t1[:], mu[:], var[:], A.mult)
                st3 = sm1.tile([TP, 3, TG], BF16, tag="st3")
                nc.vector.tensor_copy(st3[:, 0, :], var[:])
                nc.vector.tensor_copy(st3[:, 1, :], t1[:])
                nc.vector.memset(st3[:, 2, :], 1.0)
                for s3 in range(3):
                    nc.sync.dma_start(
                        bass.AP(tensor=dst3.tensor, offset=s3 * NTOK,
                                ap=[[TG, TP], [1, TG]]), st3[:, s3, :])
                return h16

            def ln_rep(lhsA, lhsB, repA, repB):
                for c in range(NCH):
                    sl = slice(CW * c, CW * (c + 1))
                    st3c = sm.tile([3, CW], BF16, tag="st3c")
                    nc.sync.dma_start(st3c[:], dst3[:, sl])
                    psA_ = psp.tile([80, CW], F32, tag="ps")
                    nc.tensor.matmul(psA_[0:D, :], lhsA[:], st3c[:], start=True, stop=True)
                    nc.scalar.activation(repA[0:D, sl], psA_[0:D, :], AF.Copy, scale=1.0)
                    psB_ = psp.tile([80, CW], F32, tag="ps")
                    nc.tensor.matmul(psB_[0:D, :], lhsB[:], st3c[:], start=True, stop=True)
                    nc.scalar.activation(repB[0:D, sl], psB_[0:D, :], AF.Copy, scale=1.0)

            # ================= layers =================
            for li in range(NL):
                # ---- LN1 -> hnpad interior ----
                h16 = ln_stats(hres_f)
                repA = slotA()
                repB = slotB()
                ln_rep(wt["lnI_A"], wt["lnI_B"], repA, repB)
                nc.vector.tensor_tensor(repA[0:D, :], h16[0:D, :], repA[0:D, :], A.mult)
                nc.vector.memset(hnpad[:, :, 0:3], 0.0)
                nc.vector.memset(hnpad[:, :, 32:35], 0.0)
                nc.vector.scalar_tensor_tensor(
                    hnpad[:, :, 3:32],
                    repA[0:D, :].rearrange("d (n p) -> d n p", n=N),
                    wt[f"c1_{li}"][:],
                    repB[0:D, :].rearrange("d (n p) -> d n p", n=N), A.add, A.subtract)
                if li == 0:
                    dbg_dump("d_hn0", hnpad[:, :, 3:32])
                # ---- conv+uproj (f/b) + z ----
                for c in range(NCH):
                    for rev in (0, 1):
                        psC = psp.tile([80, CW], F32, tag="ps")
                        for k in range(DCONV):
                            off = k if not rev else 6 - k
                            nc.tensor.matmul(
                                psC[0:DI, :], wt[f"Wconv_{li}"][:, k * DI:(k + 1) * DI],
                                hnpad[:, CH * c:CH * (c + 1), off:off + L],
                                start=(k == 0), stop=(k == DCONV - 1))
                        nc.scalar.activation(
                            (ucF if not rev else ucB)[:, CW * c:CW * (c + 1)],
                            psC[0:DI, :], AF.Silu, bias=wt[f"conv_b_{li}"][:], scale=1.0)
                    psZ = psp.tile([80, CW], F32, tag="ps")
                    nc.tensor.matmul(psZ[0:DI, :], wt[f"Wz_{li}"][:],
                                     hnpad[:, CH * c:CH * (c + 1), 3:32],
                                     start=True, stop=True)
                    nc.scalar.activation(szt[:, CW * c:CW * (c + 1)], psZ[0:DI, :], AF.Silu)
                if li == 0:
                    dbg_dump("d_uc0", ucF[:])
                # ---- xproj fused (f/b) ----
                dtF = slotA()
                dtB = slotB()
                for c in range(NCH):
                    sl = slice(CW * c, CW * (c + 1))
                    for rev in (0, 1):
                        uct = ucF if not rev else ucB
                        dtt = dtF if not rev else dtB
                        psX = psp.tile([96, CW], F32, tag="ps")
                        nc.tensor.matmul(psX[:], wt[f"Waug_{li}"][:], uct[:, sl],
                                         start=True, stop=True)
                        e32 = sm.tile([DI, CW], F32, tag="e32", bufs=1)
                        nc.scalar.activation(e32[:], psX[0:DI, :],
                                             AF.Exp, bias=wt[f"dt_b_{li}"][:], scale=1.0)
                        nc.scalar.activation(dtt[:, sl], e32[:],
                                             AF.Ln, bias=oneT[0:DI, :], scale=1.0)
                        r16c = sm.tile([DI, CW], BF16, tag="r16c", bufs=1)
                        nc.scalar.activation(r16c[:], dtt[:, sl], AF.Exp, scale=-1.0)
                        nc.sync.dma_start(ddt[rev * DI:(rev + 1) * DI, sl], r16c[:])
                        bc = sm.tile([96, CW], BF16, tag="bc", bufs=1)
                        nc.vector.tensor_copy(bc[64:96, :], psX[64:96, :])
                        nc.sync.dma_start(dBC[rev * 2 * S:(rev + 1) * 2 * S, sl],
                                          bc[64:96, :])
                if li == 0:
                    dbg_dump("d_dt0", dtF[:])
                for rev in (0, 1):
                    dtu = hsc()
                    nc.vector.tensor_tensor(
                        dtu[:], (dtF if not rev else dtB)[:],
                        (ucF if not rev else ucB)[:], A.mult)
                    nc.sync.dma_start(ddtu[rev * DI:(rev + 1) * DI, :], dtu[:])
                # ---- scan ----
                for rev in (0, 1):
                    for tix in range(NT):
                        n0 = SN * tix
                        tdtu = sc.tile([128, DD, L], BF16, tag="tdtu", bufs=1)
                        tB = sc.tile([128, S, L], BF16, tag="tB", bufs=1)
                        tC = sc.tile([128, S, L], BF16, tag="tC", bufs=1)
                        tdA = sc.tile([128, DD, S, L], BF16, tag="tdA")
                        if rev:
                            tr = sc.tile([128, DD, L], BF16, tag="tr", bufs=1)
                        for dp in range(DP):
                            fr = rev * DI + DD * dp
                            loads = [(tdtu[SN * dp:SN * (dp + 1), :, :], ddtu, fr, DD),
                                     (tB[SN * dp:SN * (dp + 1), :, :], dBC, rev * 2 * S, S),
                                     (tC[SN * dp:SN * (dp + 1), :, :], dBC, rev * 2 * S + S, S)]
                            if rev:
                                loads.append((tr[SN * dp:SN * (dp + 1), :, :], ddt, fr, DD))
                            else:
                                loads.append(
                                    (tdA[SN * dp:SN * (dp + 1), :, 0:1, :], ddt, fr, DD))
                            for dst_ap, srcd, rbase, nch in loads:
                                nc.sync.dma_start(
                                    dst_ap,
                                    bass.AP(tensor=srcd.tensor,
                                            offset=rbase * NTOK + n0 * L,
                                            ap=[[L, SN], [NTOK, nch], [1, L]]))
                        tdtu_v = tdtu[:, :, ::-1] if rev else tdtu[:]
                        tB_v = tB[:, :, ::-1] if rev else tB[:]
                        tC_v = tC[:, :, ::-1] if rev else tC[:]
                        if rev:
                            nc.scalar.activation(tdA[:, :, 0, :], tr[:, :, ::-1],
                                                 AF.Copy, scale=1.0)
                        nc.vector.tensor_tensor(tdA[:, :, 1, :], tdA[:, :, 0, :],
                                                tdA[:, :, 0, :], A.mult)
                        nc.vector.tensor_tensor(
                            tdA[:, :, 2:4, :], tdA[:, :, 0:2, :],
                            tdA[:, :, 1:2, :].broadcast_to([128, DD, 2, L]), A.mult)
                        nc.vector.tensor_tensor(
                            tdA[:, :, 4:8, :], tdA[:, :, 0:4, :],
                            tdA[:, :, 3:4, :].broadcast_to([128, DD, 4, L]), A.mult)
                        nc.vector.tensor_tensor(
                            tdA[:, :, 8:16, :], tdA[:, :, 0:8, :],
                            tdA[:, :, 7:8, :].broadcast_to([128, DD, 8, L]), A.mult)
                        nc.gpsimd.memset(tdA[:, :, :, 0:1], 0.0)
                        th = sc.tile([128, DD, S, L], BF16, tag="th", bufs=1)
                        nc.vector.tensor_tensor(
                            th[:], tdtu_v.unsqueeze(2).broadcast_to([128, DD, S, L]),
                            tB_v.unsqueeze(1).broadcast_to([128, DD, S, L]), A.mult)
                        nc.vector.tensor_tensor_scan(
                            th[:].rearrange("p a s l -> p (a s l)"),
                            tdA[:].rearrange("p a s l -> p (a s l)"),
                            th[:].rearrange("p a s l -> p (a s l)"),
                            0.0, A.mult, A.add)
                        nc.vector.tensor_tensor(
                            th[:], th[:], tC_v.unsqueeze(1).broadcast_to([128, DD, S, L]),
                            A.mult)
                        nc.vector.tensor_tensor(th[:, :, 0:8, :], th[:, :, 0:8, :],
                                                th[:, :, 8:16, :], A.add)
                        nc.vector.tensor_tensor(th[:, :, 0:4, :], th[:, :, 0:4, :],
                                                th[:, :, 4:8, :], A.add)
                        nc.vector.tensor_tensor(th[:, :, 0:2, :], th[:, :, 0:2, :],
                                                th[:, :, 2:4, :], A.add)
                        nc.vector.tensor_tensor(th[:, :, 0, :], th[:, :, 0, :],
                                                th[:, :, 1, :], A.add)
                        for dp in range(DP):
                            fr = rev * DI + DD * dp
                            nc.sync.dma_start(
                                bass.AP(tensor=dy.tensor,
                                        offset=fr * NTOK + n0 * L,
                                        ap=[[L, SN], [NTOK, DD], [1, L]]),
                                th[SN * dp:SN * (dp + 1), :, 0, :])
                # ---- gates + out_proj + residual ----
                tyF = slotA()
                tyB = slotB()
                nc.sync.dma_start(tyF[:], dy[0:DI, :])
                nc.sync.dma_start(tyB[:], dy[DI:2 * DI, :])
                if li == 0:
                    dbg_dump("d_y0", tyF[:])
                nc.vector.tensor_tensor(tyF[:], tyF[:], szt[:], A.mult)
                gyB = hsc()
                nc.vector.tensor_tensor(
                    gyB[:].rearrange("d (n l) -> d n l", n=N),
                    tyB[:].rearrange("d (n l) -> d n l", n=N)[:, :, ::-1],
                    szt[:].rearrange("d (n l) -> d n l", n=N), A.mult)
                nc.vector.tensor_tensor(ucF[:], ucF[:], szt[:], A.mult)
                nc.vector.tensor_tensor(ucB[:], ucB[:], szt[:], A.mult)
                for c in range(NCH):
                    sl = slice(CW * c, CW * (c + 1))
                    psO = psp.tile([80, CW], F32, tag="ps")
                    nc.tensor.matmul(psO[0:D, :], wt[f"Wouty_{li}"][:], tyF[:, sl],
                                     start=True, stop=False)
                    nc.tensor.matmul(psO[0:D, :], wt[f"Wouty_{li}"][:], gyB[:, sl],
                                     start=False, stop=False)
                    nc.tensor.matmul(psO[0:D, :], wt[f"Woutuc_{li}"][:], ucF[:, sl],
                                     start=False, stop=False)
                    nc.tensor.matmul(psO[0:D, :], wt[f"Woutuc_{li}"][:], ucB[:, sl],
                                     start=False, stop=True)
                    nc.vector.tensor_tensor(hres_f[:, sl], hres_f[:, sl], psO[0:D, :], A.add)
                # ---- LN2 -> new hres ----
                h16 = ln_stats(hres_f)
                repA = slotA()
                repB = slotB()
                ln_rep(wt[f"ln2A_{li}"], wt[f"ln2B_{li}"], repA, repB)
                nc.vector.tensor_tensor(repA[0:D, :], h16[0:D, :], repA[0:D, :], A.mult)
                nc.vector.tensor_tensor(hres_f[:], repA[0:D, :], repB[0:D, :], A.subtract)
                dbg_dump(f"d_hL{li}", hres_f)

            # ================= attention pool + head =================
            h16 = hsc()
            nc.scalar.activation(h16[0:D, :], hres_f, AF.Copy, scale=1.0)
            uat = slotA()
            for c in range(NCH):
                sl = slice(CW * c, CW * (c + 1))
                psA_ = psp.tile([80, CW], F32, tag="ps")
                nc.tensor.matmul(psA_[0:D, :], wt["attnW"][:], h16[0:D, sl],
                                 start=True, stop=True)
                nc.scalar.activation(uat[0:D, sl], psA_[0:D, :], AF.Tanh,
                                     bias=wt["attn_b"][:], scale=1.0)
                psSc = psp.tile([80, CW], F32, tag="ps")
                nc.tensor.matmul(psSc[0:1, :], wt["ctx"][:], uat[0:D, sl],
                                 start=True, stop=True)
                scc = sm.tile([1, CW], F32, tag="stc", bufs=1, name="scc")
                nc.vector.tensor_copy(scc[:], psSc[0:1, :])
                nc.sync.dma_start(dst2[0:1, sl], scc[:])
            ssc = sm1.tile([TP, 3, L], F32, tag="ssc")
            nc.sync.dma_start(ssc[:], bass.AP(tensor=dst2.tensor, offset=0,
                                              ap=[[3 * L, TP], [L, 3], [1, L]]))

            smax = sm1.tile([TP, 3], F32, tag="smax")
            nc.vector.tensor_reduce(smax[:], ssc[:], AX.X, A.max)
            nc.vector.tensor_tensor(
                ssc[:], ssc[:], smax[:].unsqueeze(2).broadcast_to([TP, 3, L]), A.subtract)
            nc.scalar.activation(ssc[:], ssc[:], AF.Exp, scale=1.0)
            ssum = sm1.tile([TP, 3], F32, tag="ssum")
            nc.vector.tensor_reduce(ssum[:], ssc[:], AX.X, A.add)
            nc.vector.reciprocal(ssum[:], ssum[:])
            nc.vector.tensor_tensor(
                ssc[:], ssc[:], ssum[:].unsqueeze(2).broadcast_to([TP, 3, L]), A.mult)
            asc16 = sm1.tile([TP, 3, L], BF16, tag="asc16")
            nc.vector.tensor_copy(asc16[:], ssc[:])
            nc.sync.dma_start(bass.AP(tensor=dst3.tensor, offset=0,
                                      ap=[[3 * L, TP], [L, 3], [1, L]]), asc16[:])
            onesD = sm1.tile([1, D], BF16, tag="onesD")
            nc.vector.memset(onesD[:], 1.0)
            hw16 = slotB()
            for c in range(NCH):
                sl = slice(CW * c, CW * (c + 1))
                alc = sm1.tile([1, CW], BF16, tag="alc")
                nc.sync.dma_start(alc[:], dst3[0:1, sl])
                psL = psp.tile([80, CW], F32, tag="ps")
                nc.tensor.matmul(psL[0:D, :], onesD[:], alc[:], start=True, stop=True)
                nc.vector.tensor_tensor(hw16[0:D, sl], h16[0:D, sl], psL[0:D, :], A.mult)
            cpool = sm1.tile([D, N], F32, tag="cpool")
            nc.vector.tensor_reduce(
                cpool[:], hw16[0:D, :].rearrange("d (n p) -> d n p", n=N), AX.X, A.add)
            cmean = sm1.tile([D, NB], F32, tag="cmean")
            nc.vector.tensor_reduce(
                cmean[:], cpool[:].rearrange("d (b c) -> d b c", b=NB), AX.X, A.add)
            nc.scalar.mul(cmean[:], cmean[:], 1.0 / C)
            c16 = sm1.tile([D, NB], BF16, tag="c16")
            csq16 = sm1.tile([D, NB], BF16, tag="csq16")
            nc.scalar.activation(c16[:], cmean[:], AF.Copy, scale=1.0)
            nc.scalar.square(csq16[:], cmean[:])
            psSh = psp.tile([80, CW], F32, tag="ps")
            nc.tensor.matmul(psSh[0:2, 0:NB], wt["ones2a"][:], c16[:], start=True, stop=False)
            nc.tensor.matmul(psSh[0:2, 0:NB], wt["ones2b"][:], csq16[:], start=False, stop=True)
            sAB = sm1.tile([2, NB], F32, tag="sAB")
            nc.vector.tensor_copy(sAB[:], psSh[0:2, 0:NB])
            sB0 = sm1.tile([1, NB], F32, tag="sB0")
            nc.sync.dma_start(sB0[:], sAB[1:2, :])
            hmu = sm1.tile([1, NB], F32, tag="hmu")
            hvar = sm1.tile([1, NB], F32, tag="hvar")
            hm2 = sm1.tile([1, NB], F32, tag="hm2")
            nc.scalar.mul(hmu[:], sAB[0:1, :], 1.0 / D)
            nc.scalar.mul(hvar[:], sB0[:], 1.0 / D)
            nc.vector.tensor_tensor(hm2[:], hmu[:], hmu[:], A.mult)
            nc.vector.tensor_tensor(hvar[:], hvar[:], hm2[:], A.subtract)
            nc.scalar.activation(hvar[:], hvar[:], AF.Sqrt, bias=epsT[0:1, :], scale=1.0)
            nc.vector.reciprocal(hvar[:], hvar[:])
            nc.vector.tensor_tensor(hm2[:], hmu[:], hvar[:], A.mult)
            r3 = sm1.tile([3, NB], BF16, tag="r3")
            r1b = sm1.tile([1, NB], BF16, tag="r1b")
            nc.vector.tensor_copy(r1b[:], hvar[:])
            nc.sync.dma_start(r3[0:1, :], r1b[:])
            nc.vector.tensor_copy(r1b[:], hm2[:])
            nc.sync.dma_start(r3[1:2, :], r1b[:])
            nc.vector.memset(r1b[:], 1.0)
            nc.sync.dma_start(r3[2:3, :], r1b[:])
            psRA = psp.tile([80, CW], F32, tag="ps")
            nc.tensor.matmul(psRA[0:D, 0:NB], wt["lnI_A"][:], r3[:], start=True, stop=True)
            psRB = psp.tile([80, CW], F32, tag="ps")
            nc.tensor.matmul(psRB[0:D, 0:NB], wt["lnI_B"][:], r3[:], start=True, stop=True)
            cn1 = sm1.tile([D, NB], F32, tag="cn1")
            nc.vector.tensor_tensor(cn1[:], cmean[:], psRA[0:D, 0:NB], A.mult)
            cn16 = sm1.tile([D, NB], BF16, tag="cn16")
            nc.vector.tensor_tensor(cn16[:], cn1[:], psRB[0:D, 0:NB], A.subtract)
            psH = psp.tile([80, CW], F32, tag="ps")
            nc.tensor.matmul(psH[0:NCLS, 0:NB], wt["headW"][:], cn16[:], start=True, stop=True)
            hout = sm1.tile([NCLS, NB], F32, tag="hout")
            nc.scalar.activation(hout[:], psH[0:NCLS, 0:NB], AF.Identity,
                                 bias=wt["head_bias"][:], scale=1.0)
            nc.sync.dma_start(yout[:], hout[:])
    nc.compile()
    return nc, dbg_t


# ---------------- PJRT runner (inlined) ----------------
import time as _time
import jax as _jax


def make_runner(nc, n_cores):
    from concourse import bass2jax, mybir
    bass2jax.install_neuronx_cc_hook()
    assert nc.dbg_addr is None or not nc.dbg_callbacks
    partition_name = nc.partition_id_tensor.name if nc.partition_id_tensor else None

    in_names, out_names, out_avals, zero_outs = [], [], [], []
    for alloc in nc.m.functions[0].allocations:
        if not isinstance(alloc, mybir.MemoryLocationSet):
            continue
        name = alloc.memorylocations[0].name
        if alloc.kind == "ExternalInput":
            if name != partition_name and name != (
                nc.dbg_addr.name if nc.dbg_addr else None
            ):
                in_names.append(name)
        elif alloc.kind == "ExternalOutput":
            out_names.append(name)
            np_dt = mybir.dt.np(alloc.dtype)
            out_avals.append(
                _jax.core.ShapedArray(tuple(alloc.tensor_shape), np_dt)
            )
            zero_outs.append(np.zeros(tuple(alloc.tensor_shape), np_dt))

    n_params = len(in_names)
    all_in_names = list(in_names) + list(out_names)
    if nc.dbg_addr is not None:
        all_in_names.append(nc.dbg_addr.name)
    if partition_name is not None:
        all_in_names.append(partition_name)

    def _body(*args):
        operands = list(args)
        if nc.dbg_addr is not None:
            operands.append(np.zeros((1, 2), np.uint32))
        if partition_name is not None:
            operands.append(bass2jax.partition_id_tensor())
        outs = bass2jax._bass_exec_p.bind(
            *operands,
            out_avals=tuple(out_avals),
            in_names=tuple(all_in_names),
            out_names=tuple(out_names),
            lowering_input_output_aliases=(),
            sim_require_finite=True,
            sim_require_nnan=True,
            nc=nc,
        )
        return tuple(outs)

    if n_cores == 1:
        jitted = _jax.jit(_body, keep_unused=True)

        def run(in_map):
            args = [np.asarray(in_map[n]) for n in in_names] + zero_outs
            outs = jitted(*args)
            return {n: np.asarray(o) for n, o in zip(out_names, outs)}
    else:
        from jax.sharding import Mesh, PartitionSpec
        from jax.experimental.shard_map import shard_map

        devices = _jax.devices()[:n_cores]
        mesh = Mesh(np.asarray(devices), ("core",))
        n_outs = len(out_names)
        in_specs = (PartitionSpec("core"),) * (n_params + n_outs)
        out_specs = (PartitionSpec("core"),) * n_outs
        jitted = _jax.jit(
            shard_map(_body, mesh=mesh, in_specs=in_specs, out_specs=out_specs,
                      check_rep=False),
            keep_unused=True,
        )

        def run(in_maps):
            concat_in = [
                np.concatenate([np.asarray(m[n]) for m in in_maps], axis=0)
                for n in in_names
            ]
            concat_zero = [
                np.zeros((n_cores * z.shape[0], *z.shape[1:]), z.dtype)
                for z in zero_outs
            ]
            outs = jitted(*concat_in, *concat_zero)
            out_np = [np.asarray(o) for o in outs]
            result = []
            for c in range(n_cores):
                result.append({
                    n: o[c * z.shape[0]:(c + 1) * z.shape[0]]
                    for n, o, z in zip(out_names, out_np, zero_outs)
                })
            return result

    return run


def time_runner(run, arg, iters=10, warmup=3):
    for _ in range(warmup):
        run(arg)
    times = []
    for _ in range(iters):
        t0 = _time.perf_counter()
        run(arg)
        t1 = _time.perf_counter()
        times.append(t1 - t0)
    return min(times), float(np.median(times))


_CACHE = {}


def kernel(**inputs):
    if "runner" not in _CACHE:
        nc, _ = build()
        _CACHE["runner"] = make_runner(nc, NCORES)
    run = _CACHE["runner"]
    p = prep_weights(inputs)
    x = np.asarray(inputs["x"], np.float32)
    in_maps = []
    for ci in range(NCORES):
        m = dict(p)
        m["x"] = x[ci * NB:(ci + 1) * NB]
        in_maps.append(m)
    res = run(in_maps)
    return np.concatenate([r["y"].T for r in res], 0).astype(np.float32)

